# revision 58
# baseline (speedup 1.0000x reference)
"""Trainium2 Bass kernel for nn_AttnBlock (B=4, C=64, H=W=64 self-attention block).

Sharding: 8 cores = (batch b in 0..3) x (query-half in 0..1). Each core
computes attention for 2048 query tokens of one batch element against all
4096 key/value tokens of that element. Weights are replicated.

Layout strategy (per core):
  - x_b packed per-core as [128, 2048] bf16: each partition half holds
    1024 query-token columns first, then 1024 non-query columns (attention
    is permutation-invariant over keys, so k/v just iterate packed order;
    this removes the old duplicated-xq DMA block)
  - k = WkT.T @ x  -> [64, 4096]; q = WqT.T @ xq -> [64, 2048] (duplicated
    on both partition halves via a [wq|wq] lhsT)
  - v in [token, channel] layout [128, 32mt, 65] with a trailing ones
    column (gives the softmax denominator for free in the P.V matmul)
  - scoresT[m, n] = k^T q computed per 128-key-tile into PSUM groups,
    exp()'d by ScalarE directly PSUM->SBUF (scale=1/8, no max subtraction:
    scores are ~N(0, 8^2) so exp(s/8) is far from overflow)
  - htT_aug[65, n] = sum_m v_aug[m, :] pT[m, n]  (row 64 = denominator)
  - out[c, n] = xres[c, n] + (Wp @ htT[0:64]) * (1/denominator) broadcast
    (partition-broadcast of the reciprocal row on GpSimd)

PE scheduling notes:
  - score matmuls pair even/odd key tiles on disjoint PE row halves
    (K=64 each) so pairs run concurrently
  - q projections pair the same way (issue order q0|q2, q1|q3)
  - k projections split each x chunk into an even-key-tile matmul
    (psum rows 0-63, PE col groups 0,1) and an odd-tile matmul (out
    base 64, col groups 2,3): the two run concurrently AND the odd
    tiles evacuate directly into k2hi partitions 64-127, removing the
    old SBUF->SBUF replication DMA chain from the scores prologue.
    Concurrent matmuls must never write the same psum bank (that
    wedges the device), hence the b0-lo|b1-hi / b2-lo|b0-hi banking.
  - v projections stay sequential: half-alternating issue wedges the
    device for those short-N matmuls in every arrangement tried
  - the tail out-projection (rows 0-63) is woven in right after an
    odd score matmul (rows 64-127) so the two overlap
  - tail chunk PAIRS (0,2) and (1,3) stage into one [128, 512] tile
    (first chunk rows 0-63, second rows 64-127) and ship as one DMA,
    halving per-partition output-DMA bytes. This pairing matches the
    packed-xin layout (chunks 0,1 queries on rows 0-63; chunks 2,3 on
    64-127), so the residual add reads bf16 x straight from xin and
    no separate residual input exists at all.
  - a dummy exp right after the ones-memset preloads the ScalarE Exp
    table set off the critical path (~2.7us on single-shot runs)

Single input: "xin" [128, 2368] bf16 = per-core packed x + replicated
weights (592KB/core vs 1.38MB for the original kernel).
"""

import os
import sys

for _p in ("/opt/trn_rl_repo",):
    if _p not in sys.path:
        sys.path.insert(0, _p)

import numpy as np

import concourse.bacc as bacc
import concourse.bass as bass
import concourse.mybir as mybir
import concourse.tile as tile
from concourse.bass_utils import run_bass_kernel_spmd

B, C, H, W = 4, 64, 64, 64
N = H * W            # 4096 tokens
HALF = N // 2        # 2048 query tokens per core
CHUNK = 512          # query-chunk (psum bank width in fp32)
NCHUNKS = HALF // CHUNK   # 4
MT = N // 128        # 32 key tiles of 128 tokens
# v_sb slot per key tile (identity: half-alternating v projection pairing
# crashes the device -- concurrent short-N MMs writing psum; see notes)
VSLOT = {mt: mt for mt in range(32)}

# packed [128, XIN2] input: per partition-half -> [x-cols | weights].
# Per-core column order puts the core's 2048 query tokens at packed cols
# 0-1023 of BOTH partition halves (lo rows: queries 0-1023, hi rows:
# queries 1024-2047), with the non-query tokens at packed cols 1024-2047.
# Attention is permutation-invariant over keys, so k/v simply iterate the
# packed order. This removes the old duplicated-xq DMA block (256KB/core).
# weights block: [wq|wq] (128 cols, doubled for duplicated-q production)
# then wk, wv, wp (64 each) -> 320 cols, replicated on both halves
XIN2 = N // 2 + 5 * C   # 2368 columns per partition row

F32 = mybir.dt.float32
BF16 = mybir.dt.bfloat16

# matmul operand dtype. fp32/f32r matmuls are "self-loading" (walrus
# generates the LDWEIGHTS internally) and can encode only ONE semaphore
# wait -- Tile routinely needs 2+, so 4-byte matmuls fail codegen with
# "Too many sync wait commands". bf16 keeps LDW/MM as separate
# instructions and streams 1 col/cycle through the PE.
DT_MM = BF16

LAST_RESULTS = None  # test harness can inspect exec_time_ns etc.

# bisection knobs for HW timing experiments (never set in graded runs)
SKIP_EXP = os.environ.get("ATTN_SKIP_EXP") == "1"
SKIP_PV = os.environ.get("ATTN_SKIP_PV") == "1"
SKIP_SCORES = os.environ.get("ATTN_SKIP_SCORES") == "1"


def _build_nc(loop_iters=None, skip=None):
    """loop_iters: if set, wrap the whole kernel body in a hardware loop --
    used only for wall-clock timing (amortizes host/axon dispatch).
    skip: iterable of {"exp","scores","pv"} -- timing-only ablations."""
    if skip is None:
        skip = set()
        if SKIP_EXP:
            skip.add("exp")
        if SKIP_PV:
            skip.add("pv")
        if SKIP_SCORES:
            skip.add("scores")
    skip = set(skip)
    nc = bacc.Bacc()

    # Packed 128-partition inputs for full DMA bandwidth:
    #   xin128[p, :]: for p<64 (channel c=p) columns hold
    #     [x chunks 0-3 | xq chunks 0-1 | wq wk] and for p>=64 (c=p-64)
    #     [x chunks 4-7 | xq chunks 2-3 | wv wp].
    xin_d = nc.dram_tensor("xin", [128, XIN2], BF16, kind="ExternalInput")
    out_d = nc.dram_tensor("out", [C, HALF], F32, kind="ExternalOutput")

    EXP = mybir.ActivationFunctionType.Exp
    MUL = mybir.AluOpType.mult
    ADD = mybir.AluOpType.add

    with (
        tile.TileContext(nc) as tc,
        tc.tile_pool(name="main", bufs=1) as mpool,
        tc.tile_pool(name="work", bufs=3) as wpool,
        tc.tile_pool(name="psum", bufs=1, space="PSUM") as ppool,
    ):
        import contextlib
        loop_cm = (
            tc.For_i(0, loop_iters, 1, hint_engines=(
                mybir.EngineType.PE, mybir.EngineType.Activation,
                mybir.EngineType.DVE, mybir.EngineType.SP))
            if loop_iters else contextlib.nullcontext()
        )
        with loop_cm:
            xin = mpool.tile([128, XIN2], BF16, name="xin")
            # weights first (tiny), then the query-token columns so q
            # production starts early, then the rest
            nc.sync.dma_start(xin[:, N // 2 :], xin_d[:, N // 2 :])
            nc.sync.dma_start(xin[:, : N // 4], xin_d[:, : N // 4])
            nc.sync.dma_start(xin[:, N // 4 : N // 2], xin_d[:, N // 4 : N // 2])

            def xt_cols(c0, w):
                """x[:, c0:c0+w] as a [64, w] AP (w must stay in one 2048-col half)."""
                half, off = divmod(c0, N // 2)
                assert off + w <= N // 2
                return xin[64 * half : 64 * half + 64, off : off + w]

            def xq_cols(c0, w):
                # query chunk c0 lives at packed cols 0-1023 of its half
                half, off = divmod(c0, HALF // 2)
                assert off + w <= HALF // 2
                return xin[64 * half : 64 * half + 64, off : off + w]

            def w_g(g, half=0):
                # weights are replicated on both partition halves so lhsT can
                # match the rhs's base partition (PE rows = SBUF partitions).
                # g=0 -> [wq|wq] (128 wide, for duplicated-q production);
                # g=1..3 -> wk/wv/wp (64 wide)
                base = N // 2
                if g == 0:
                    return xin[64 * half : 64 * half + 64, base : base + 2 * C]
                off = base + (g + 1) * C
                return xin[64 * half : 64 * half + 64, off : off + C]

            # residual reads the bf16 query columns straight from xin:
            # tail chunk pairs are (0,2) and (1,3), so each chunk's staging
            # partition half (rows 0-63 for ch<2, 64-127 for ch>=2) matches
            # where its query columns live in the packed layout
            xres_cols = xq_cols

            wq, wk, wv, wp = w_g(0), w_g(1), w_g(2), w_g(3)

            q_dup = mpool.tile([128, HALF], DT_MM, name="q_dup")
            k_sb = mpool.tile([C, N // 2], DT_MM, name="k_sb")  # even tiles only
            v_sb = mpool.tile([128, MT, C + 1], DT_MM, name="v_sb")  # +ones col
            pT = mpool.tile([128, MT, CHUNK], DT_MM, name="pT")
            nc.vector.memset(v_sb[:, :, C : C + 1], 1.0)
            # warm the Exp table set off the critical path: the first real
            # exp would otherwise stall ~2.7us on ACT_TABLE_LOAD in a
            # single-shot run. Depends only on the memset above.
            warm = mpool.tile([1, 1], F32, name="warm")
            nc.scalar.activation(
                warm[:],
                v_sb[0:1, 0:1, C : C + 1].rearrange("p a b -> p (a b)"),
                EXP, bias=0.0, scale=1.0,
            )
            sc_fake = None
            if "scores" in skip:
                sc_fake = mpool.tile([128, 3, CHUNK], F32, name="sc_fake")
                nc.vector.memset(sc_fake[:], 0.5)
            if "exp" in skip:
                nc.vector.memset(pT[:, :, 0:1], 1.0)

            # ---- q / k / v projections ----
            # Issue order alternates PE row halves (h0 at rows 0-63, h1 at
            # 64-127) so consecutive MMs occupy disjoint row groups and run
            # concurrently: q0|q2, q1|q3, k0|k4 ... k3|k7, v0|v16 ... v15|v31.
            # PSUM tags: s = [128,3,512] double-buffered groups (6 banks),
            # pvtail = PV accumulator / tail projection (2 banks).
            ps_q = ppool.tile([128, 3, CHUNK], F32, name="ps_q", tag="s", bufs=2)
            for j, qc in enumerate((0, 2, 1)):  # halves 0,1,0
                nc.tensor.matmul(
                    ps_q[:, j, :], w_g(0, qc // 2), xq_cols(qc * CHUNK, CHUNK),
                    start=True, stop=True,
                )
            ps_q2 = ppool.tile([128, CHUNK], F32, name="ps_q2", tag="pvtail", bufs=2)
            nc.tensor.matmul(
                ps_q2[:, :], w_g(0, 1), xq_cols(3 * CHUNK, CHUNK),
                start=True, stop=True,
            )
            # ps_q slots (q0, q2, q1): chunk 0 evacuates alone on DVE (it
            # gates phase-0 scores); chunk 2 goes to ScalarE in parallel
            nc.vector.tensor_copy(q_dup[:, 0:CHUNK], ps_q[:, 0, :])
            nc.scalar.copy(q_dup[:, 2 * CHUNK : 3 * CHUNK], ps_q[:, 1, :])
            nc.vector.tensor_copy(q_dup[:, CHUNK : 2 * CHUNK], ps_q[:, 2, :])
            nc.vector.tensor_copy(q_dup[:, 3 * CHUNK :], ps_q2[:])

            # k production: each 512-col x chunk projects in two matmuls --
            # EVEN key tiles to psum rows 0-63 (PE col groups 0,1) and ODD
            # key tiles to rows 64-127 (out base 64 -> col groups 2,3), so
            # the pair runs concurrently. Odd tiles evacuate straight into
            # k2hi's partitions 64-127: the old SBUF->SBUF replication DMA
            # chain is gone and chunk-0 scores are ready after the first
            # evacuation. Two chunks share one 3-bank psum tile; concurrent
            # pair outputs always land in distinct banks
            # (b0-lo | b1-hi, then b2-lo | b0-hi-cols).
            k2hi = mpool.tile([128, MT // 2, 128], DT_MM, name="k2hi")

            def xt_eo(c, odd):
                # the two even (or odd) 128-col key tiles of x chunk c
                half, off = divmod(c * CHUNK, N // 2)
                return xin[
                    64 * half : 64 * half + 64, off : off + CHUNK
                ].rearrange("p (a b) -> p a b", b=128)[:, odd::2, :]

            for cp in range(4):
                cA, cB = 2 * cp, 2 * cp + 1
                ps_k = ppool.tile([128, 3, CHUNK], F32, name="ps_k", tag="s", bufs=2)
                nc.tensor.matmul(
                    ps_k[0:C, 0, 0:256], w_g(1, cA // 4), xt_eo(cA, 0),
                    start=True, stop=True,
                )
                nc.tensor.matmul(
                    ps_k[C:128, 1, 0:256], w_g(1, cA // 4), xt_eo(cA, 1),
                    start=True, stop=True,
                )
                nc.tensor.matmul(
                    ps_k[0:C, 2, 0:256], w_g(1, cB // 4), xt_eo(cB, 0),
                    start=True, stop=True,
                )
                nc.tensor.matmul(
                    ps_k[C:128, 0, 256:512], w_g(1, cB // 4), xt_eo(cB, 1),
                    start=True, stop=True,
                )
                nc.scalar.copy(
                    k_sb[:, cA * 256 : cA * 256 + 256], ps_k[0:C, 0, 0:256]
                )
                nc.vector.tensor_copy(
                    k2hi[C:128, 2 * cA : 2 * cA + 2, :],
                    ps_k[C:128, 1, 0:256].rearrange("p (a b) -> p a b", b=128),
                )
                nc.scalar.copy(
                    k_sb[:, cB * 256 : cB * 256 + 256], ps_k[0:C, 2, 0:256]
                )
                nc.vector.tensor_copy(
                    k2hi[C:128, 2 * cB : 2 * cB + 2, :],
                    ps_k[C:128, 0, 256:512].rearrange("p (a b) -> p a b", b=128),
                )

            # v projections stay in sequential mt order: half-alternating
            # issue (like q/k above) wedges the device for these short-N
            # matmuls, in every psum-bank arrangement tried.
            ps_v = ppool.tile([128, 3, 8, C], F32, name="ps_v", tag="s", bufs=2)
            for mt in range(24):
                nc.tensor.matmul(
                    ps_v[:, mt // 8, mt % 8, :],
                    xt_cols(mt * 128, 128), w_g(2, mt // 16),
                    start=True, stop=True,
                )
            ps_v2 = ppool.tile([128, 8, C], F32, name="ps_v2", tag="pvtail", bufs=2)
            for mt in range(24, MT):
                nc.tensor.matmul(
                    ps_v2[:, mt - 24, :], xt_cols(mt * 128, 128), w_g(2, 1),
                    start=True, stop=True,
                )
            nc.vector.tensor_copy(
                v_sb[:, 0:24, :C].rearrange("p (a b) c -> p a b c", a=3), ps_v[:]
            )
            nc.scalar.copy(v_sb[:, 24:MT, :C], ps_v2[:])

            # ---- attention over query chunks (software-pipelined) ----
            # scores+exp for chunk ch overlap P.V for chunk ch-1: PV matmuls are
            # interleaved between score groups on the PE queue so ScalarE (the
            # bottleneck: 8.4M exps) never starves. One uniform score tag with
            # bufs=2 rotates globally -- no pipeline drain at chunk boundaries.
            groups = []
            mt0 = 0
            while mt0 < MT:
                gs = min(3, MT - mt0)
                groups.append((mt0, gs))
                mt0 += gs

            state = {}

            def emit_tail_pre(ch):
                """DVE/GpSimd part of the tail: evacuate PV, 1/denominator."""
                pv = state.pop("pv")
                htT = wpool.tile([C, CHUNK], DT_MM, name="htT", tag="htT")
                nc.vector.tensor_copy(htT[:], pv[:C])
                # denominator evacuates on ScalarE, in parallel with the DVE
                # htT copy: the denom->recip->broadcast chain gates the tail
                denom = wpool.tile([1, CHUNK], F32, name="denom", tag="denom")
                nc.scalar.copy(denom[:], pv[C : C + 1, :])

                recip = wpool.tile([1, CHUNK], F32, name="recip", tag="recip")
                nc.vector.reciprocal(recip[:], denom[:])

                # broadcast 1/denominator across all 128 partitions on GpSimd
                # (idle engine; keeps the reciprocal exact fp32); the tail
                # uses rows 0-63 for even chunks, 64-127 for odd ones
                rb = wpool.tile([128, CHUNK], F32, name="rb", tag="rb")
                nc.gpsimd.partition_broadcast(rb[:], recip[:])
                state["tail"] = (ch, htT, rb)

            def emit_tail_post():
                """PE projection + residual + store; issued one score-group
                after emit_tail_pre so the PE queue never stalls on DVE.
                Chunk pairs stage into one [128, 512] tile (even chunk on
                rows 0-63, odd on 64-127) and ship as a single DMA, halving
                per-partition output-DMA bytes."""
                ch, htT, rb = state.pop("tail")
                h, pid = ch // 2, ch % 2  # staging partition half, pair id
                sl = slice(64 * h, 64 * h + 64)
                # project the un-normalized ht; the 1/denominator scale
                # commutes with the (linear) projection, applied at the end.
                ps_o = ppool.tile([128, CHUNK], F32, name="ps_o", tag="pvtail", bufs=2)
                nc.tensor.matmul(ps_o[sl, :], w_g(3, 0), htT[:], start=True, stop=True)

                if h == 0:
                    state[f"opair{pid}"] = wpool.tile(
                        [128, CHUNK], F32, name="out_sb", tag="out_sb"
                    )
                opair = state[f"opair{pid}"]
                nc.vector.tensor_tensor(opair[sl, :], ps_o[sl, :], rb[sl, :], MUL)
                nc.vector.tensor_tensor(
                    opair[sl, :], opair[sl, :], xres_cols(ch * CHUNK, CHUNK), ADD
                )
                if h == 1:
                    state.pop(f"opair{pid}")
                    # rows 0-63 -> chunk pid, rows 64-127 -> chunk pid+2
                    nc.sync.dma_start(
                        out_d[:, :].rearrange(
                            "c (a t b) -> a t c b", a=2, b=CHUNK
                        )[:, pid : pid + 1, :, :],
                        opair[:],
                    )

            for ph in range(NCHUNKS + 1):
                for gi, (m0, gs) in enumerate(groups):
                    # tail projection (rows 0-63, cols 0-63) is woven in
                    # right after group 1's first score MM (kt3: rows
                    # 64-127, all cols) so the two run concurrently; on
                    # the drain phase there are no scores, emit it here.
                    do_tail = "tail" in state and gi == 1
                    if do_tail and (ph >= NCHUNKS or "scores" in skip):
                        emit_tail_post()
                        do_tail = False
                    if ph > 0:
                        # P.V slice for the previous chunk (same mts whose pT
                        # this group's exp will overwrite right after)
                        if gi == 0:
                            state["pv"] = ppool.tile(
                                [C + 1, CHUNK], F32, name="ps_pv", tag="pvtail", bufs=2
                            )
                        for mt in range(m0, m0 + gs):
                            if "pv" in skip and mt not in (0, MT - 1):
                                continue
                            nc.tensor.matmul(
                                state["pv"][:], v_sb[:, VSLOT[mt], :], pT[:, mt, :],
                                start=(mt == 0), stop=(mt == MT - 1),
                            )
                    if ph < NCHUNKS:
                        if "scores" not in skip:
                            ps_s = ppool.tile([128, 3, CHUNK], F32, name="ps_s", tag="s", bufs=2)
                            for j in range(gs):
                                mt = m0 + j
                                # even key-tiles contract on PE rows 0-63, odd
                                # ones on rows 64-127 -> pairs run concurrently
                                if mt % 2 == 0:
                                    et = mt // 2
                                    lhsT = k_sb[:, et * 128 : (et + 1) * 128]
                                    rhs = q_dup[0:C, ph * CHUNK : (ph + 1) * CHUNK]
                                else:
                                    lhsT = k2hi[64:128, mt // 2, :]
                                    rhs = q_dup[C:128, ph * CHUNK : (ph + 1) * CHUNK]
                                nc.tensor.matmul(
                                    ps_s[:, j, :], lhsT, rhs, start=True, stop=True,
                                )
                                if do_tail and j == 0:
                                    emit_tail_post()
                                    do_tail = False
                        else:
                            ps_s = sc_fake
                        # exp((k^T q) / sqrt(C)) straight PSUM -> SBUF
                        if "exp" not in skip:
                            nc.scalar.activation(
                                pT[:, m0 : m0 + gs, :], ps_s[:, :gs, :], EXP,
                                bias=0.0, scale=0.125,
                            )
                if ph > 0:
                    emit_tail_pre(ph - 1)
            emit_tail_post()

    nc.compile()
    return nc


_NC = None


def _get_nc():
    global _NC
    if _NC is None:
        _NC = _build_nc()
    return _NC


def _make_in_maps(x, Wq, Wk, Wv, Wp):
    import ml_dtypes
    x = np.ascontiguousarray(x, dtype=np.float32)
    Wq, Wk, Wv, Wp = (np.asarray(w, dtype=np.float32) for w in (Wq, Wk, Wv, Wp))
    wall = np.concatenate(
        [Wq.T, Wq.T, Wk.T, Wv.T, Wp.T], axis=1
    ).astype(np.float32)  # [c_in, 5*c_out] = [64, 320]

    in_maps = []
    for core in range(8):
        b, half = core >> 1, core & 1
        xb = x[b].reshape(C, N)
        xh = xb[:, half * HALF : (half + 1) * HALF]           # query half
        xo = xb[:, (1 - half) * HALF : (2 - half) * HALF]     # other half
        # per-core packed column order: queries first on both partition
        # halves (see XIN2 comment); key iteration order is permuted per
        # core, which attention is invariant to
        lo = np.concatenate([xh[:, : HALF // 2], xo[:, : HALF // 2], wall], axis=1)
        hi = np.concatenate([xh[:, HALF // 2 :], xo[:, HALF // 2 :], wall], axis=1)
        xin = np.concatenate([lo, hi], axis=0).astype(ml_dtypes.bfloat16)
        in_maps.append({"xin": np.ascontiguousarray(xin)})

    return in_maps


def kernel(x, Wq, Wk, Wv, Wp):
    global LAST_RESULTS
    nc = _get_nc()
    in_maps = _make_in_maps(x, Wq, Wk, Wv, Wp)
    res = run_bass_kernel_spmd(nc, in_maps, list(range(8)))
    LAST_RESULTS = res

    y = np.empty((B, C, N), dtype=np.float32)
    for core in range(8):
        b, half = core >> 1, core & 1
        y[b, :, half * HALF : (half + 1) * HALF] = res.results[core]["out"]
    return y.reshape(B, C, H, W)

